# revision 10
# baseline (speedup 1.0000x reference)
"""Trainium2 Bass kernel for nn_ConvNet3 (conv(1->32, k=(3,2500), s=(1,1250)) +
relu + 1x1 conv + relu + scalar Elman RNN over T=99 + sigmoid).

Strategy (pure data parallel, batch sharded 2-per-core across 8 cores):

  * The big conv is decomposed on non-overlapping 1250-wide input stripes:
    window t of the conv covers stripes (t, t+1) and channel rows (c-1,c,c+1),
    so  y[oc,c,t] = sum_{kh,j} <w[oc,kh,j,:], xb[c+kh-1, t+j, :]>  with
    xb[c,s,:] the 1250-wide stripe s of (replicate-padded) channel row c.
    Per stripe we compute all 192 = (kh,j,oc) dot products as a matmul:
    lhsT = W [K=128 (10 chunks, 1250 zero-padded), M=96 (x2 halves)],
    rhs = X [128, positions].  Dense matmul, no duplicated input data.
    K chunks are padded 125->128 because a DMA's SDMA-engine spray is
    largest-divisor(partitions)<=16: 125 rows -> 5 engines, 128 -> 16.
  * Positions are STRIPE-major: p = s*66 + cp (s = stripe, cp = padded
    channel row).  This makes z[:, t] complete as stripes finish, so the
    serial RNN runs pipelined behind the conv instead of after it.
  * P[(kh,j,oc), p] partials accumulate in PSUM over the 10 K-chunks, then
    are copied (cast bf16) to SBUF with the per-row-block position shift
    sh = j*66 + kh baked in, so all 6 y-terms align at one view.
  * y = relu(sum of 2 partition-group views of P + conv_b): 2 accumulating
    TensorE selector matmuls (lhsT = 96x32 identity stack) per t-chunk of 8,
    yp[oc, t, c] in PSUM; ScalarE relu evacuates.
  * z = relu(w2 . y + b2): TensorE matvec (M=1) + ScalarE relu (bf16), then
    DMA to a DRAM staging buffer zstage[t, lane] with lane = b*64 + c for
    this core's two batches.
  * RNN: 16-row chunks of zstage are transpose-DMA'd to [128 lanes, 16 t];
    each step is ONE ScalarE activation h = tanh(whh*h + a[:,t]) on all 128
    (batch,channel) lanes, a = wih*z + (b_ih+b_hh) precomputed per chunk.
    Steps interleave with conv windows as their z becomes ready.
  * Scalar network parameters are baked into the program as immediates.

x loads are one conv-window each ([128, 10, 512] bf16 = 1.28 MB, 10 KB
contiguous per partition), alternating the qSP HWDGE queue and the gpsimd
SWDGE queue so the two descriptor streams keep all 16 SDMA engines fed.
"""

import os

import numpy as np
import ml_dtypes

bf16 = ml_dtypes.bfloat16

# Problem shape
B, C, W = 16, 64, 125000
KH, KW, SW, OC = 3, 2500, 1250, 32
T = (W - KW) // SW + 1  # 99
S = W // SW             # 100 stripes per row
Q, KI = 10, 128         # contraction 1250 = Q chunks of 125, zero-padded to 128
KR = 125                # real contraction rows per chunk
CP = C + 2              # replicate-padded channel rows
MB = CP * S             # 6600 stripe-major positions (s, cp) per batch
WPOS = 512              # position window
NW = 13                 # windows per batch
MBP = NW * WPOS         # 6656 padded positions
NCORES = 8
BPC = B // NCORES       # 2 batches per core
HALF = 96               # M per conv matmul; 2 halves cover 6*OC=192
# row blocks of 32 within each half: (kh,j) -> shift j*66 + kh
SHIFTS = [[0, 66, 1], [67, 2, 68]]
TCH = 8                 # t-steps per z chunk (8*64 = 512 psum cols)
NCH = 13                # z chunks (12*8 + 3 = 99)
TG = 16                 # t-steps per RNN transpose group
NG = 7                  # groups (6*16 + 3 = 99)

LAST_RESULTS = None  # BassKernelResults of the most recent run (for test.py)


def _build_nc(wih, whh, btot, b2):
    """Build the single-core Bass program (shared SPMD across all 8 cores)."""
    import concourse.bass as bass  # noqa: F401
    import concourse.mybir as mybir
    import concourse.tile as tile
    from concourse import bacc

    f32 = mybir.dt.float32
    b16 = mybir.dt.bfloat16
    AF = mybir.ActivationFunctionType

    nc = bacc.Bacc("TRN2", target_bir_lowering=False, debug=False)

    x_d = nc.dram_tensor("x", [BPC * NW, KI, Q * WPOS], b16, kind="ExternalInput")
    w_d = nc.dram_tensor("w", [KI, Q, 2 * HALF], b16, kind="ExternalInput")
    w2_d = nc.dram_tensor("w2", [OC, 1], b16, kind="ExternalInput")
    cb_d = nc.dram_tensor("cb", [OC, 1], f32, kind="ExternalInput")
    h0_d = nc.dram_tensor("h0", [BPC * C, 1], f32, kind="ExternalInput")
    id3_d = nc.dram_tensor("id3", [HALF, OC], b16, kind="ExternalInput")
    out_d = nc.dram_tensor("out", [BPC * C, 1], f32, kind="ExternalOutput")
    # One staging tensor per RNN transpose group: disjoint tensors keep the
    # DRAM dependency tracking from serializing store(k+1) behind the
    # transpose-read of group k (whole-tensor WAR).
    zstage = [nc.dram_tensor(f"zstage{g}", [TG, BPC * C], b16)
              for g in range(NG)]

    with tile.TileContext(nc) as tc:
        with (
            tc.tile_pool(name="consts", bufs=1) as consts,
            tc.tile_pool(name="xp", bufs=2) as xpool,
            tc.tile_pool(name="pbig", bufs=1) as pbig,
            tc.tile_pool(name="ya", bufs=3) as yapool,
            tc.tile_pool(name="zf", bufs=3) as zfpool,
            tc.tile_pool(name="rnn", bufs=2) as rnnpool,
            tc.tile_pool(name="pP", bufs=1, space="PSUM") as pP,
            tc.tile_pool(name="pyy", bufs=2, space="PSUM") as pyy,
            tc.tile_pool(name="pz", bufs=2, space="PSUM") as pz,
        ):
            wt = consts.tile([KI, Q, 2 * HALF], b16)
            nc.sync.dma_start(out=wt[:, :, :], in_=w_d[:, :, :])
            w2t = consts.tile([OC, 1], b16)
            nc.sync.dma_start(out=w2t[:, :], in_=w2_d[:, :])
            cbt = consts.tile([OC, 1], f32)
            nc.sync.dma_start(out=cbt[:, :], in_=cb_d[:, :])
            b2t = consts.tile([1, 1], f32)
            nc.vector.memset(b2t[:, :], float(b2))
            id3 = consts.tile([HALF, OC], b16)
            nc.sync.dma_start(out=id3[:, :], in_=id3_d[:, :])
            h = consts.tile([BPC * C, 1], f32, tag="h", name="h")
            nc.sync.dma_start(out=h[:, :], in_=h0_d[:, :])

            # P partial-product store per (batch, half): [96, 6656] bf16.
            P = [
                [pbig.tile([HALF, 101 * CP], b16, tag=f"P{b}{hh}", name=f"P{b}{hh}")
                 for hh in range(2)]
                for b in range(BPC)
            ]

            def emit_zchunk(k):
                """z[t0:t0+tn, :] for both batches -> zstage."""
                t0 = TCH * k
                tn = min(TCH, T - t0)
                for b in range(BPC):
                    Pr = [P[b][hh][:, :].rearrange("p (t c) -> p t c", c=CP)
                          for hh in range(2)]
                    yp = pyy.tile([OC, TCH, C], mybir.dt.float32, tag="yp", name="yp")
                    for hh in range(2):
                        nc.tensor.matmul(
                            yp[:, :tn, :], id3[:, :],
                            Pr[hh][0:HALF, t0:t0 + tn, 0:C],
                            start=(hh == 0), stop=(hh == 1))
                    ya = yapool.tile([OC, TCH, C], b16, tag="ya", name="ya")
                    nc.scalar.activation(
                        out=ya[:, :tn, :], in_=yp[:, :tn, :],
                        func=AF.Relu, bias=cbt[:, 0:1], scale=1.0)
                    zp = pz.tile([1, TCH * C], mybir.dt.float32, tag="zp", name="zp")
                    nc.tensor.matmul(
                        zp[0:1, :tn * C], w2t[:, 0:1],
                        ya[:, :tn, :].rearrange("p t c -> p (t c)"),
                        start=True, stop=True)
                    zf = zfpool.tile([1, TCH * C], b16, tag="zf", name="zf")
                    nc.scalar.activation(out=zf[0:1, :tn * C], in_=zp[0:1, :tn * C],
                                         func=AF.Relu, bias=b2t[0:1, 0:1], scale=1.0)
                    g, r0 = divmod(t0, TG)
                    nc.scalar.dma_start(
                        out=zstage[g][r0:r0 + tn, b * C:(b + 1) * C],
                        in_=zf[0:1, :tn * C].rearrange("p (t c) -> p t c", c=C))

            def emit_rnn_group(g):
                """Transpose-load z rows [16g, 16g+16) and run the RNN steps."""
                t0 = TG * g
                tn = min(TG, T - t0)
                z2 = rnnpool.tile([BPC * C, TG], b16, tag="z2", name=f"z2{g}")
                nc.scalar.dma_start_transpose(
                    out=z2[:, :], in_=zstage[g][:, :])
                a = rnnpool.tile([BPC * C, TG], mybir.dt.float32, tag="a",
                                 name=f"a{g}")
                nc.scalar.activation(out=a[:, :tn], in_=z2[:, :tn], func=AF.Copy,
                                     bias=float(btot), scale=float(wih))
                for t in range(tn):
                    nc.scalar.activation(out=h[:, :], in_=h[:, :],
                                         func=AF.Tanh, bias=a[:, t:t + 1],
                                         scale=float(whh))

            dma_eng = [nc.sync, nc.gpsimd]
            kdone = 0
            gdone = 0
            for w in range(NW):
                for b in range(BPC):
                    xt = xpool.tile([KI, Q, WPOS], b16, tag=f"xt{b}")
                    dma_eng[(w * BPC + b) % 2].dma_start(
                        out=xt[:, :, :],
                        in_=x_d[b * NW + w, :, :].rearrange("k (q m) -> k q m", q=Q))
                    for hh in range(2):
                        acc = pP.tile([HALF, WPOS], mybir.dt.float32,
                                      tag=f"acc{b}{hh}", name="acc")
                        for q in range(Q):
                            nc.tensor.matmul(
                                acc[:, :],
                                wt[:, q, HALF * hh:HALF * (hh + 1)],
                                xt[:, q, :],
                                start=(q == 0), stop=(q == Q - 1))
                        # Evacuate with per-row-block position shifts baked in.
                        for ul in range(3):
                            sh = SHIFTS[hh][ul]
                            d0 = w * WPOS - sh
                            s0, n = 0, WPOS
                            if d0 < 0:
                                s0, n, d0 = -d0, WPOS + d0, 0
                            nc.vector.tensor_copy(
                                out=P[b][hh][32 * ul:32 * (ul + 1), d0:d0 + n],
                                in_=acc[32 * ul:32 * (ul + 1), s0:s0 + n])
                # z chunks / RNN groups whose positions are now covered
                pos_done = WPOS * (w + 1)
                while kdone < NCH and (
                    (TCH * kdone + min(TCH, T - TCH * kdone) - 1) * CP
                    + (C - 1) + SHIFTS[1][2] < pos_done
                ):
                    emit_zchunk(kdone)
                    kdone += 1
                # one extra chunk of slack so the transpose's RAW wait on the
                # staging stores is already satisfied when ScalarE reaches it
                while gdone < NG and kdone >= min(2 * gdone + 3, NCH - 1 if gdone == 5 else NCH):
                    emit_rnn_group(gdone)
                    gdone += 1
            while kdone < NCH:
                emit_zchunk(kdone)
                kdone += 1
            while gdone < NG:
                emit_rnn_group(gdone)
                gdone += 1

            osb = rnnpool.tile([BPC * C, 1], mybir.dt.float32, tag="o", name="o")
            nc.scalar.activation(out=osb[:, :], in_=h[:, :], func=AF.Sigmoid)
            nc.scalar.dma_start(out=out_d[:, :], in_=osb[:, :])

    nc.compile()
    return nc


def _prep_inputs(inputs):
    """Host-side layout prep (pad/reshape/transpose/cast only) -> per-core maps."""
    x = np.asarray(inputs["x"], np.float32)
    conv_w = np.asarray(inputs["conv_w"], np.float32)
    conv_b = np.asarray(inputs["conv_b"], np.float32)
    conv2_w = np.asarray(inputs["conv2_w"], np.float32)
    h0 = np.asarray(inputs["h0"], np.float32)

    xp = np.pad(x[:, 0], ((0, 0), (1, 1), (0, 0)), mode="edge")  # [B, CP, W]
    A = xp.reshape(B, CP, S, Q, KR).transpose(0, 2, 1, 3, 4)     # [B, S, CP, Q, KR]
    A = A.reshape(B, MB, Q, KR)
    A = np.pad(A, ((0, 0), (0, MBP - MB), (0, 0), (0, KI - KR)))
    A = A.reshape(B, NW, WPOS, Q, KI).transpose(0, 1, 4, 3, 2)   # [B, NW, KI, Q, WPOS]
    Xh = A.astype(bf16)  # contiguous copy

    Wh = (np.pad(conv_w[:, 0].reshape(OC, KH, 2, Q, KR),
                 ((0, 0), (0, 0), (0, 0), (0, 0), (0, KI - KR)))
          .transpose(4, 3, 1, 2, 0).reshape(KI, Q, 2 * HALF).astype(bf16))
    Wh = np.ascontiguousarray(Wh)
    w2h = np.ascontiguousarray(conv2_w[0, :, 0, 0].reshape(OC, 1).astype(bf16))
    cbh = np.ascontiguousarray(conv_b.reshape(OC, 1).astype(np.float32))
    id3h = np.ascontiguousarray(np.tile(np.eye(OC, dtype=np.float32), (3, 1)).astype(bf16))

    in_maps = []
    for cid in range(NCORES):
        xc = np.ascontiguousarray(Xh[BPC * cid:BPC * (cid + 1)]).reshape(
            BPC * NW, KI, Q * WPOS)
        h0c = np.repeat(h0[0, BPC * cid:BPC * (cid + 1), 0], C).reshape(
            BPC * C, 1).astype(np.float32)
        in_maps.append({"x": xc, "w": Wh, "w2": w2h, "cb": cbh, "h0": h0c,
                        "id3": id3h})

    scalars = dict(
        wih=float(np.asarray(inputs["w_ih"])[0, 0]),
        whh=float(np.asarray(inputs["w_hh"])[0, 0]),
        btot=float(np.asarray(inputs["b_ih"])[0] + np.asarray(inputs["b_hh"])[0]),
        b2=float(np.asarray(inputs["conv2_b"])[0]),
    )
    return in_maps, scalars


def kernel(**inputs):
    global LAST_RESULTS
    from concourse.bass_utils import run_bass_kernel_spmd

    in_maps, sc = _prep_inputs(inputs)
    nc = _build_nc(sc["wih"], sc["whh"], sc["btot"], sc["b2"])

    trace = bool(os.environ.get("KERNEL_TRACE"))
    res = run_bass_kernel_spmd(nc, in_maps, core_ids=list(range(NCORES)),
                               trace=trace)
    LAST_RESULTS = res
    outs = [r["out"].reshape(BPC, C) for r in res.results]
    return np.concatenate(outs, axis=0).astype(np.float32)


# revision 12
# speedup vs baseline: 1.0795x; 1.0795x over previous
"""Trainium2 Bass kernel for nn_ConvNet3 (conv(1->32, k=(3,2500), s=(1,1250)) +
relu + 1x1 conv + relu + scalar Elman RNN over T=99 + sigmoid).

Strategy (pure data parallel, batch sharded 2-per-core across 8 cores):

  * The big conv is decomposed on non-overlapping 1250-wide input stripes:
    window t of the conv covers stripes (t, t+1) and channel rows (c-1,c,c+1),
    so  y[oc,c,t] = sum_{kh,j} <w[oc,kh,j,:], xb[c+kh-1, t+j, :]>  with
    xb[c,s,:] the 1250-wide stripe s of (replicate-padded) channel row c.
    Per stripe we compute all 192 = (kh,j,oc) dot products as a matmul:
    lhsT = W [K=128 (10 chunks, 1250 zero-padded), M=96 (x2 halves)],
    rhs = X [128, positions].  Dense matmul, no duplicated input data.
    K chunks are padded 125->128 because a DMA's SDMA-engine spray is
    largest-divisor(partitions)<=16: 125 rows -> 5 engines, 128 -> 16.
  * Positions are STRIPE-major: p = s*66 + cp (s = stripe, cp = padded
    channel row).  This makes z[:, t] complete as stripes finish, so the
    serial RNN runs pipelined behind the conv instead of after it.
  * P[(kh,j,oc), p] partials accumulate in PSUM over the 10 K-chunks, then
    are copied (cast bf16) to SBUF with the per-row-block position shift
    sh = j*66 + kh baked in, so all 6 y-terms align at one view.
  * y = relu(sum of 2 partition-group views of P + conv_b): 2 accumulating
    TensorE selector matmuls (lhsT = 96x32 identity stack) per t-chunk of 8,
    yp[oc, t, c] in PSUM; ScalarE relu evacuates.
  * z = relu(w2 . y + b2): TensorE matvec (M=1) + ScalarE relu (bf16), then
    DMA to a DRAM staging buffer zstage[t, lane] with lane = b*64 + c for
    this core's two batches.
  * RNN: 16-row chunks of zstage are transpose-DMA'd to [128 lanes, 16 t];
    each step is ONE ScalarE activation h = tanh(whh*h + a[:,t]) on all 128
    (batch,channel) lanes, a = wih*z + (b_ih+b_hh) precomputed per chunk.
    Steps interleave with conv windows as their z becomes ready.
  * Scalar network parameters are baked into the program as immediates.

x loads are one conv-window each ([128, 10, 512] bf16 = 1.28 MB, 10 KB
contiguous per partition), alternating the qSP HWDGE queue and the gpsimd
SWDGE queue so the two descriptor streams keep all 16 SDMA engines fed.
"""

import os

import numpy as np
import ml_dtypes

bf16 = ml_dtypes.bfloat16

# Problem shape
B, C, W = 16, 64, 125000
KH, KW, SW, OC = 3, 2500, 1250, 32
T = (W - KW) // SW + 1  # 99
S = W // SW             # 100 stripes per row
Q, KI = 10, 128         # contraction 1250 = Q chunks of 125, zero-padded to 128
KR = 125                # real contraction rows per chunk
CP = C + 2              # replicate-padded channel rows
MB = CP * S             # 6600 stripe-major positions (s, cp) per batch
WPOS = 512              # position window
NW = 13                 # windows per batch
MBP = NW * WPOS         # 6656 padded positions
NCORES = 8
BPC = B // NCORES       # 2 batches per core
HALF = 96               # M per conv matmul; 2 halves cover 6*OC=192
# row blocks of 32 within each half: (kh,j) -> shift j*66 + kh
SHIFTS = [[0, 66, 1], [67, 2, 68]]
TCH = 8                 # t-steps per z chunk (8*64 = 512 psum cols)
NCH = 13                # z chunks (12*8 + 3 = 99)
TG = 16                 # t-steps per RNN transpose group
NG = 7                  # groups (6*16 + 3 = 99)

LAST_RESULTS = None  # BassKernelResults of the most recent run (for test.py)


def _build_nc(wih, whh, btot, b2):
    """Build the single-core Bass program (shared SPMD across all 8 cores)."""
    import concourse.bass as bass  # noqa: F401
    import concourse.mybir as mybir
    import concourse.tile as tile
    from concourse import bacc

    f32 = mybir.dt.float32
    b16 = mybir.dt.bfloat16
    AF = mybir.ActivationFunctionType

    nc = bacc.Bacc("TRN2", target_bir_lowering=False, debug=False)

    x_d = nc.dram_tensor("x", [BPC * NW, KI, Q * WPOS], b16, kind="ExternalInput")
    w_d = nc.dram_tensor("w", [KI, Q, 2 * HALF], b16, kind="ExternalInput")
    w2_d = nc.dram_tensor("w2", [OC, 1], b16, kind="ExternalInput")
    cb_d = nc.dram_tensor("cb", [OC, 1], f32, kind="ExternalInput")
    h0_d = nc.dram_tensor("h0", [BPC * C, 1], f32, kind="ExternalInput")
    id3_d = nc.dram_tensor("id3", [HALF, OC], b16, kind="ExternalInput")
    out_d = nc.dram_tensor("out", [BPC * C, 1], f32, kind="ExternalOutput")
    # One staging tensor per RNN transpose group: disjoint tensors keep the
    # DRAM dependency tracking from serializing store(k+1) behind the
    # transpose-read of group k (whole-tensor WAR).
    zstage = [nc.dram_tensor(f"zstage{g}", [TG, BPC * C], b16)
              for g in range(NG)]

    with tile.TileContext(nc) as tc:
        with (
            tc.tile_pool(name="consts", bufs=1) as consts,
            tc.tile_pool(name="xp", bufs=3) as xpool,
            tc.tile_pool(name="pbig", bufs=1) as pbig,
            tc.tile_pool(name="ya", bufs=3) as yapool,
            tc.tile_pool(name="zf", bufs=3) as zfpool,
            tc.tile_pool(name="rnn", bufs=2) as rnnpool,
            tc.tile_pool(name="pP", bufs=1, space="PSUM") as pP,
            tc.tile_pool(name="pyy", bufs=2, space="PSUM") as pyy,
            tc.tile_pool(name="pz", bufs=2, space="PSUM") as pz,
        ):
            wt = consts.tile([KI, Q, 2 * HALF], b16)
            nc.sync.dma_start(out=wt[:, :, :], in_=w_d[:, :, :])
            w2t = consts.tile([OC, 1], b16)
            nc.sync.dma_start(out=w2t[:, :], in_=w2_d[:, :])
            cbt = consts.tile([OC, 1], f32)
            nc.sync.dma_start(out=cbt[:, :], in_=cb_d[:, :])
            b2t = consts.tile([1, 1], f32)
            nc.vector.memset(b2t[:, :], float(b2))
            id3 = consts.tile([HALF, OC], b16)
            nc.sync.dma_start(out=id3[:, :], in_=id3_d[:, :])
            h = consts.tile([BPC * C, 1], f32, tag="h", name="h")
            nc.sync.dma_start(out=h[:, :], in_=h0_d[:, :])

            # P partial-product store per (batch, half): [96, 6656] bf16.
            P = [
                [pbig.tile([HALF, 101 * CP], b16, tag=f"P{b}{hh}", name=f"P{b}{hh}")
                 for hh in range(2)]
                for b in range(BPC)
            ]

            def emit_zchunk(k):
                """z[t0:t0+tn, :] for both batches -> zstage."""
                t0 = TCH * k
                tn = min(TCH, T - t0)
                for b in range(BPC):
                    Pr = [P[b][hh][:, :].rearrange("p (t c) -> p t c", c=CP)
                          for hh in range(2)]
                    yp = pyy.tile([OC, TCH, C], mybir.dt.float32, tag="yp", name="yp")
                    for hh in range(2):
                        nc.tensor.matmul(
                            yp[:, :tn, :], id3[:, :],
                            Pr[hh][0:HALF, t0:t0 + tn, 0:C],
                            start=(hh == 0), stop=(hh == 1))
                    ya = yapool.tile([OC, TCH, C], b16, tag="ya", name="ya")
                    nc.scalar.activation(
                        out=ya[:, :tn, :], in_=yp[:, :tn, :],
                        func=AF.Relu, bias=cbt[:, 0:1], scale=1.0)
                    zp = pz.tile([1, TCH * C], mybir.dt.float32, tag="zp", name="zp")
                    nc.tensor.matmul(
                        zp[0:1, :tn * C], w2t[:, 0:1],
                        ya[:, :tn, :].rearrange("p t c -> p (t c)"),
                        start=True, stop=True)
                    zf = zfpool.tile([1, TCH * C], b16, tag="zf", name="zf")
                    nc.scalar.activation(out=zf[0:1, :tn * C], in_=zp[0:1, :tn * C],
                                         func=AF.Relu, bias=b2t[0:1, 0:1], scale=1.0)
                    g, r0 = divmod(t0, TG)
                    nc.scalar.dma_start(
                        out=zstage[g][r0:r0 + tn, b * C:(b + 1) * C],
                        in_=zf[0:1, :tn * C].rearrange("p (t c) -> p t c", c=C))

            def emit_rnn_group(g):
                """Transpose-load z rows [16g, 16g+16) and run the RNN steps."""
                t0 = TG * g
                tn = min(TG, T - t0)
                z2 = rnnpool.tile([BPC * C, TG], b16, tag="z2", name=f"z2{g}")
                nc.scalar.dma_start_transpose(
                    out=z2[:, :], in_=zstage[g][:, :])
                a = rnnpool.tile([BPC * C, TG], mybir.dt.float32, tag="a",
                                 name=f"a{g}")
                nc.scalar.activation(out=a[:, :tn], in_=z2[:, :tn], func=AF.Copy,
                                     bias=float(btot), scale=float(wih))
                for t in range(tn):
                    nc.scalar.activation(out=h[:, :], in_=h[:, :],
                                         func=AF.Tanh, bias=a[:, t:t + 1],
                                         scale=float(whh))

            # x loads all on the gpsimd SWDGE queue: SWDGE completions tick
            # separate DMASW sem lanes, so the z-staging / transpose DMAs on
            # the shared DMAHW lanes never inherit waits on demand-paced
            # x loads (false cross-lane deps serialized the whole pipeline).
            kdone = 0
            gdone = 0
            for w in range(NW):
                for b in range(BPC):
                    xt = xpool.tile([KI, Q, WPOS], b16, tag=f"xt{b}")
                    nc.gpsimd.dma_start(
                        out=xt[:, :, :],
                        in_=x_d[b * NW + w, :, :].rearrange("k (q m) -> k q m", q=Q))
                    for hh in range(2):
                        acc = pP.tile([HALF, WPOS], mybir.dt.float32,
                                      tag=f"acc{b}{hh}", name="acc")
                        for q in range(Q):
                            nc.tensor.matmul(
                                acc[:, :],
                                wt[:, q, HALF * hh:HALF * (hh + 1)],
                                xt[:, q, :],
                                start=(q == 0), stop=(q == Q - 1))
                        # Evacuate with per-row-block position shifts baked in.
                        for ul in range(3):
                            sh = SHIFTS[hh][ul]
                            d0 = w * WPOS - sh
                            s0, n = 0, WPOS
                            if d0 < 0:
                                s0, n, d0 = -d0, WPOS + d0, 0
                            nc.vector.tensor_copy(
                                out=P[b][hh][32 * ul:32 * (ul + 1), d0:d0 + n],
                                in_=acc[32 * ul:32 * (ul + 1), s0:s0 + n])
                # z chunks / RNN groups whose positions are now covered
                pos_done = WPOS * (w + 1)
                while kdone < NCH and (
                    (TCH * kdone + min(TCH, T - TCH * kdone) - 1) * CP
                    + (C - 1) + SHIFTS[1][2] < pos_done
                ):
                    emit_zchunk(kdone)
                    kdone += 1
                # one extra chunk of slack so the transpose's RAW wait on the
                # staging stores is already satisfied when ScalarE reaches it
                while gdone < NG and kdone >= min(2 * gdone + 3, NCH - 1 if gdone == 5 else NCH):
                    emit_rnn_group(gdone)
                    gdone += 1
            while kdone < NCH:
                emit_zchunk(kdone)
                kdone += 1
            while gdone < NG:
                emit_rnn_group(gdone)
                gdone += 1

            osb = rnnpool.tile([BPC * C, 1], mybir.dt.float32, tag="o", name="o")
            nc.scalar.activation(out=osb[:, :], in_=h[:, :], func=AF.Sigmoid)
            nc.scalar.dma_start(out=out_d[:, :], in_=osb[:, :])

    nc.compile()
    return nc


def _prep_inputs(inputs):
    """Host-side layout prep (pad/reshape/transpose/cast only) -> per-core maps."""
    x = np.asarray(inputs["x"], np.float32)
    conv_w = np.asarray(inputs["conv_w"], np.float32)
    conv_b = np.asarray(inputs["conv_b"], np.float32)
    conv2_w = np.asarray(inputs["conv2_w"], np.float32)
    h0 = np.asarray(inputs["h0"], np.float32)

    xp = np.pad(x[:, 0], ((0, 0), (1, 1), (0, 0)), mode="edge")  # [B, CP, W]
    A = xp.reshape(B, CP, S, Q, KR).transpose(0, 2, 1, 3, 4)     # [B, S, CP, Q, KR]
    A = A.reshape(B, MB, Q, KR)
    A = np.pad(A, ((0, 0), (0, MBP - MB), (0, 0), (0, KI - KR)))
    A = A.reshape(B, NW, WPOS, Q, KI).transpose(0, 1, 4, 3, 2)   # [B, NW, KI, Q, WPOS]
    Xh = A.astype(bf16)  # contiguous copy

    Wh = (np.pad(conv_w[:, 0].reshape(OC, KH, 2, Q, KR),
                 ((0, 0), (0, 0), (0, 0), (0, 0), (0, KI - KR)))
          .transpose(4, 3, 1, 2, 0).reshape(KI, Q, 2 * HALF).astype(bf16))
    Wh = np.ascontiguousarray(Wh)
    w2h = np.ascontiguousarray(conv2_w[0, :, 0, 0].reshape(OC, 1).astype(bf16))
    cbh = np.ascontiguousarray(conv_b.reshape(OC, 1).astype(np.float32))
    id3h = np.ascontiguousarray(np.tile(np.eye(OC, dtype=np.float32), (3, 1)).astype(bf16))

    in_maps = []
    for cid in range(NCORES):
        xc = np.ascontiguousarray(Xh[BPC * cid:BPC * (cid + 1)]).reshape(
            BPC * NW, KI, Q * WPOS)
        h0c = np.repeat(h0[0, BPC * cid:BPC * (cid + 1), 0], C).reshape(
            BPC * C, 1).astype(np.float32)
        in_maps.append({"x": xc, "w": Wh, "w2": w2h, "cb": cbh, "h0": h0c,
                        "id3": id3h})

    scalars = dict(
        wih=float(np.asarray(inputs["w_ih"])[0, 0]),
        whh=float(np.asarray(inputs["w_hh"])[0, 0]),
        btot=float(np.asarray(inputs["b_ih"])[0] + np.asarray(inputs["b_hh"])[0]),
        b2=float(np.asarray(inputs["conv2_b"])[0]),
    )
    return in_maps, scalars


def kernel(**inputs):
    global LAST_RESULTS
    from concourse.bass_utils import run_bass_kernel_spmd

    in_maps, sc = _prep_inputs(inputs)
    nc = _build_nc(sc["wih"], sc["whh"], sc["btot"], sc["b2"])

    trace = bool(os.environ.get("KERNEL_TRACE"))
    res = run_bass_kernel_spmd(nc, in_maps, core_ids=list(range(NCORES)),
                               trace=trace)
    LAST_RESULTS = res
    outs = [r["out"].reshape(BPC, C) for r in res.results]
    return np.concatenate(outs, axis=0).astype(np.float32)


# revision 14
# speedup vs baseline: 1.0926x; 1.0121x over previous
"""Trainium2 Bass kernel for nn_ConvNet3 (conv(1->32, k=(3,2500), s=(1,1250)) +
relu + 1x1 conv + relu + scalar Elman RNN over T=99 + sigmoid).

Strategy (pure data parallel, batch sharded 2-per-core across 8 cores):

  * The big conv is decomposed on non-overlapping 1250-wide input stripes:
    window t of the conv covers stripes (t, t+1) and channel rows (c-1,c,c+1),
    so  y[oc,c,t] = sum_{kh,j} <w[oc,kh,j,:], xb[c+kh-1, t+j, :]>  with
    xb[c,s,:] the 1250-wide stripe s of (replicate-padded) channel row c.
    Per stripe we compute all 192 = (kh,j,oc) dot products as a matmul:
    lhsT = W [K=128 (10 chunks, 1250 zero-padded), M=96 (x2 halves)],
    rhs = X [128, positions].  Dense matmul, no duplicated input data.
    K chunks are padded 125->128 because a DMA's SDMA-engine spray is
    largest-divisor(partitions)<=16: 125 rows -> 5 engines, 128 -> 16.
  * Positions are STRIPE-major: p = s*66 + cp (s = stripe, cp = padded
    channel row).  This makes z[:, t] complete as stripes finish, so the
    serial RNN runs pipelined behind the conv instead of after it.
  * P[(kh,j,oc), p] partials accumulate in PSUM over the 10 K-chunks, then
    are copied (cast bf16) to SBUF with the per-row-block position shift
    sh = j*66 + kh baked in, so all 6 y-terms align at one view.
  * y = relu(sum of 2 partition-group views of P + conv_b): 2 accumulating
    TensorE selector matmuls (lhsT = 96x32 identity stack) per t-chunk of 8,
    yp[oc, t, c] in PSUM; ScalarE relu evacuates.
  * z = relu(w2 . y + b2): TensorE matvec (M=1) + ScalarE relu (bf16), then
    DMA to a DRAM staging buffer zstage[t, lane] with lane = b*64 + c for
    this core's two batches.
  * RNN: 16-row chunks of zstage are transpose-DMA'd to [128 lanes, 16 t];
    each step is ONE ScalarE activation h = tanh(whh*h + a[:,t]) on all 128
    (batch,channel) lanes, a = wih*z + (b_ih+b_hh) precomputed per chunk.
    Steps interleave with conv windows as their z becomes ready.
  * Scalar network parameters are baked into the program as immediates.

x loads are one conv-window each ([128, 10, 512] bf16 = 1.28 MB, 10 KB
contiguous per partition), alternating the qSP HWDGE queue and the gpsimd
SWDGE queue so the two descriptor streams keep all 16 SDMA engines fed.
"""

import os

import numpy as np
import ml_dtypes

bf16 = ml_dtypes.bfloat16

# Problem shape
B, C, W = 16, 64, 125000
KH, KW, SW, OC = 3, 2500, 1250, 32
T = (W - KW) // SW + 1  # 99
S = W // SW             # 100 stripes per row
Q, KI = 10, 128         # contraction 1250 = Q chunks of 125, zero-padded to 128
KR = 125                # real contraction rows per chunk
CP = C + 2              # replicate-padded channel rows
MB = CP * S             # 6600 stripe-major positions (s, cp) per batch
WPOS = 512              # position window
NW = 13                 # windows per batch
MBP = NW * WPOS         # 6656 padded positions
NCORES = 8
BPC = B // NCORES       # 2 batches per core
HALF = 96               # M per conv matmul; 2 halves cover 6*OC=192
# row blocks of 32 within each half: (kh,j) -> shift j*66 + kh
SHIFTS = [[0, 66, 1], [67, 2, 68]]
TCH = 8                 # t-steps per z chunk (8*64 = 512 psum cols)
NCH = 13                # z chunks (12*8 + 3 = 99)
TG = 16                 # t-steps per RNN transpose group
NG = 7                  # groups (6*16 + 3 = 99)

LAST_RESULTS = None  # BassKernelResults of the most recent run (for test.py)


def _build_nc(wih, whh, btot, b2):
    """Build the single-core Bass program (shared SPMD across all 8 cores)."""
    import concourse.bass as bass  # noqa: F401
    import concourse.mybir as mybir
    import concourse.tile as tile
    from concourse import bacc

    f32 = mybir.dt.float32
    b16 = mybir.dt.bfloat16
    AF = mybir.ActivationFunctionType

    nc = bacc.Bacc("TRN2", target_bir_lowering=False, debug=False)

    x_d = nc.dram_tensor("x", [BPC * NW, KI, Q * WPOS], b16, kind="ExternalInput")
    w_d = nc.dram_tensor("w", [KI, Q, 2 * HALF], b16, kind="ExternalInput")
    w2_d = nc.dram_tensor("w2", [OC, 1], b16, kind="ExternalInput")
    cb_d = nc.dram_tensor("cb", [OC, 1], f32, kind="ExternalInput")
    h0_d = nc.dram_tensor("h0", [BPC * C, 1], f32, kind="ExternalInput")
    id3_d = nc.dram_tensor("id3", [HALF, OC], b16, kind="ExternalInput")
    out_d = nc.dram_tensor("out", [BPC * C, 1], f32, kind="ExternalOutput")
    # One staging tensor per RNN transpose group: disjoint tensors keep the
    # DRAM dependency tracking from serializing store(k+1) behind the
    # transpose-read of group k (whole-tensor WAR).
    zstage = [nc.dram_tensor(f"zstage{g}", [TG, BPC * C], b16)
              for g in range(NG)]

    with tile.TileContext(nc) as tc:
        with (
            tc.tile_pool(name="consts", bufs=1) as consts,
            tc.tile_pool(name="xp", bufs=3) as xpool,
            tc.tile_pool(name="pbig", bufs=1) as pbig,
            tc.tile_pool(name="ya", bufs=3) as yapool,
            tc.tile_pool(name="zf", bufs=3) as zfpool,
            tc.tile_pool(name="rnn", bufs=2) as rnnpool,
            tc.tile_pool(name="pP", bufs=1, space="PSUM") as pP,
            tc.tile_pool(name="pyy", bufs=2, space="PSUM") as pyy,
            tc.tile_pool(name="pz", bufs=2, space="PSUM") as pz,
        ):
            wt = consts.tile([KI, Q, 2 * HALF], b16)
            nc.sync.dma_start(out=wt[:, :, :], in_=w_d[:, :, :])
            w2t = consts.tile([OC, 1], b16)
            nc.sync.dma_start(out=w2t[:, :], in_=w2_d[:, :])
            cbt = consts.tile([OC, 1], f32)
            nc.sync.dma_start(out=cbt[:, :], in_=cb_d[:, :])
            b2t = consts.tile([1, 1], f32)
            nc.vector.memset(b2t[:, :], float(b2))
            id3 = consts.tile([HALF, OC], b16)
            nc.sync.dma_start(out=id3[:, :], in_=id3_d[:, :])
            h = consts.tile([BPC * C, 1], f32, tag="h", name="h")
            nc.sync.dma_start(out=h[:, :], in_=h0_d[:, :])

            # P partial-product store per (batch, half): [96, 6656] bf16.
            P = [
                [pbig.tile([HALF, 101 * CP], b16, tag=f"P{b}{hh}", name=f"P{b}{hh}")
                 for hh in range(2)]
                for b in range(BPC)
            ]

            def emit_zchunk(k):
                """z[t0:t0+tn, :] for both batches -> zstage."""
                t0 = TCH * k
                tn = min(TCH, T - t0)
                for b in range(BPC):
                    Pr = [P[b][hh][:, :].rearrange("p (t c) -> p t c", c=CP)
                          for hh in range(2)]
                    yp = pyy.tile([OC, TCH, C], mybir.dt.float32, tag="yp", name="yp")
                    for hh in range(2):
                        nc.tensor.matmul(
                            yp[:, :tn, :], id3[:, :],
                            Pr[hh][0:HALF, t0:t0 + tn, 0:C],
                            start=(hh == 0), stop=(hh == 1))
                    ya = yapool.tile([OC, TCH, C], b16, tag="ya", name="ya")
                    nc.scalar.activation(
                        out=ya[:, :tn, :], in_=yp[:, :tn, :],
                        func=AF.Relu, bias=cbt[:, 0:1], scale=1.0)
                    zp = pz.tile([1, TCH * C], mybir.dt.float32, tag="zp", name="zp")
                    nc.tensor.matmul(
                        zp[0:1, :tn * C], w2t[:, 0:1],
                        ya[:, :tn, :].rearrange("p t c -> p (t c)"),
                        start=True, stop=True)
                    zf = zfpool.tile([1, TCH * C], b16, tag="zf", name="zf")
                    nc.scalar.activation(out=zf[0:1, :tn * C], in_=zp[0:1, :tn * C],
                                         func=AF.Relu, bias=b2t[0:1, 0:1], scale=1.0)
                    g, r0 = divmod(t0, TG)
                    nc.scalar.dma_start(
                        out=zstage[g][r0:r0 + tn, b * C:(b + 1) * C],
                        in_=zf[0:1, :tn * C].rearrange("p (t c) -> p t c", c=C))

            pending_steps = []  # (a_tile, t) tanh steps not yet emitted

            def emit_rnn_group(g):
                """Transpose-load z rows [16g, 16g+16); steps drain later."""
                t0 = TG * g
                tn = min(TG, T - t0)
                z2 = rnnpool.tile([BPC * C, TG], b16, tag="z2", name=f"z2{g}")
                nc.scalar.dma_start_transpose(
                    out=z2[:, :], in_=zstage[g][:, :])
                a = rnnpool.tile([BPC * C, TG], mybir.dt.float32, tag="a",
                                 name=f"a{g}")
                nc.scalar.activation(out=a[:, :tn], in_=z2[:, :tn], func=AF.Copy,
                                     bias=float(btot), scale=float(wih))
                for t in range(tn):
                    pending_steps.append((a, t))

            def drain_steps(n):
                """Emit up to n serial tanh steps; spreading them keeps the
                chunk relus from queueing behind a long chain on ScalarE."""
                for _ in range(min(n, len(pending_steps))):
                    a, t = pending_steps.pop(0)
                    nc.scalar.activation(out=h[:, :], in_=h[:, :],
                                         func=AF.Tanh, bias=a[:, t:t + 1],
                                         scale=float(whh))

            # x loads all on the gpsimd SWDGE queue: SWDGE completions tick
            # separate DMASW sem lanes, so the z-staging / transpose DMAs on
            # the shared DMAHW lanes never inherit waits on demand-paced
            # x loads (false cross-lane deps serialized the whole pipeline).
            kdone = 0
            gdone = 0
            for w in range(NW):
                for b in range(BPC):
                    xt = xpool.tile([KI, Q, WPOS], b16, tag=f"xt{b}")
                    nc.gpsimd.dma_start(
                        out=xt[:, :, :],
                        in_=x_d[b * NW + w, :, :].rearrange("k (q m) -> k q m", q=Q))
                    for hh in range(2):
                        acc = pP.tile([HALF, WPOS], mybir.dt.float32,
                                      tag=f"acc{b}{hh}", name="acc")
                        for q in range(Q):
                            nc.tensor.matmul(
                                acc[:, :],
                                wt[:, q, HALF * hh:HALF * (hh + 1)],
                                xt[:, q, :],
                                start=(q == 0), stop=(q == Q - 1))
                        # Evacuate with per-row-block position shifts baked in.
                        for ul in range(3):
                            sh = SHIFTS[hh][ul]
                            d0 = w * WPOS - sh
                            s0, n = 0, WPOS
                            if d0 < 0:
                                s0, n, d0 = -d0, WPOS + d0, 0
                            nc.vector.tensor_copy(
                                out=P[b][hh][32 * ul:32 * (ul + 1), d0:d0 + n],
                                in_=acc[32 * ul:32 * (ul + 1), s0:s0 + n])
                # z chunks / RNN groups whose positions are now covered
                pos_done = WPOS * (w + 1)
                while kdone < NCH and (
                    (TCH * kdone + min(TCH, T - TCH * kdone) - 1) * CP
                    + (C - 1) + SHIFTS[1][2] < pos_done
                ):
                    emit_zchunk(kdone)
                    kdone += 1
                    drain_steps(5)
                # one extra chunk of slack so the transpose's RAW wait on the
                # staging stores is already satisfied when ScalarE reaches it
                while gdone < NG and kdone >= min(2 * gdone + 3, NCH - 1 if gdone == 5 else NCH):
                    emit_rnn_group(gdone)
                    gdone += 1
                drain_steps(3)
            while kdone < NCH:
                emit_zchunk(kdone)
                kdone += 1
                drain_steps(5)
            while gdone < NG:
                emit_rnn_group(gdone)
                gdone += 1
            drain_steps(len(pending_steps))

            osb = rnnpool.tile([BPC * C, 1], mybir.dt.float32, tag="o", name="o")
            nc.scalar.activation(out=osb[:, :], in_=h[:, :], func=AF.Sigmoid)
            nc.scalar.dma_start(out=out_d[:, :], in_=osb[:, :])

    nc.compile()
    return nc


def _prep_inputs(inputs):
    """Host-side layout prep (pad/reshape/transpose/cast only) -> per-core maps."""
    x = np.asarray(inputs["x"], np.float32)
    conv_w = np.asarray(inputs["conv_w"], np.float32)
    conv_b = np.asarray(inputs["conv_b"], np.float32)
    conv2_w = np.asarray(inputs["conv2_w"], np.float32)
    h0 = np.asarray(inputs["h0"], np.float32)

    xp = np.pad(x[:, 0], ((0, 0), (1, 1), (0, 0)), mode="edge")  # [B, CP, W]
    A = xp.reshape(B, CP, S, Q, KR).transpose(0, 2, 1, 3, 4)     # [B, S, CP, Q, KR]
    A = A.reshape(B, MB, Q, KR)
    A = np.pad(A, ((0, 0), (0, MBP - MB), (0, 0), (0, KI - KR)))
    A = A.reshape(B, NW, WPOS, Q, KI).transpose(0, 1, 4, 3, 2)   # [B, NW, KI, Q, WPOS]
    Xh = A.astype(bf16)  # contiguous copy

    Wh = (np.pad(conv_w[:, 0].reshape(OC, KH, 2, Q, KR),
                 ((0, 0), (0, 0), (0, 0), (0, 0), (0, KI - KR)))
          .transpose(4, 3, 1, 2, 0).reshape(KI, Q, 2 * HALF).astype(bf16))
    Wh = np.ascontiguousarray(Wh)
    w2h = np.ascontiguousarray(conv2_w[0, :, 0, 0].reshape(OC, 1).astype(bf16))
    cbh = np.ascontiguousarray(conv_b.reshape(OC, 1).astype(np.float32))
    id3h = np.ascontiguousarray(np.tile(np.eye(OC, dtype=np.float32), (3, 1)).astype(bf16))

    in_maps = []
    for cid in range(NCORES):
        xc = np.ascontiguousarray(Xh[BPC * cid:BPC * (cid + 1)]).reshape(
            BPC * NW, KI, Q * WPOS)
        h0c = np.repeat(h0[0, BPC * cid:BPC * (cid + 1), 0], C).reshape(
            BPC * C, 1).astype(np.float32)
        in_maps.append({"x": xc, "w": Wh, "w2": w2h, "cb": cbh, "h0": h0c,
                        "id3": id3h})

    scalars = dict(
        wih=float(np.asarray(inputs["w_ih"])[0, 0]),
        whh=float(np.asarray(inputs["w_hh"])[0, 0]),
        btot=float(np.asarray(inputs["b_ih"])[0] + np.asarray(inputs["b_hh"])[0]),
        b2=float(np.asarray(inputs["conv2_b"])[0]),
    )
    return in_maps, scalars


def kernel(**inputs):
    global LAST_RESULTS
    from concourse.bass_utils import run_bass_kernel_spmd

    in_maps, sc = _prep_inputs(inputs)
    nc = _build_nc(sc["wih"], sc["whh"], sc["btot"], sc["b2"])

    trace = bool(os.environ.get("KERNEL_TRACE"))
    res = run_bass_kernel_spmd(nc, in_maps, core_ids=list(range(NCORES)),
                               trace=trace)
    LAST_RESULTS = res
    outs = [r["out"].reshape(BPC, C) for r in res.results]
    return np.concatenate(outs, axis=0).astype(np.float32)
